# revision 28
# baseline (speedup 1.0000x reference)
import sys

sys.path.insert(0, "/opt/trn_rl_repo")

import numpy as np
import ml_dtypes

import concourse.bass as bass
import concourse.bacc as bacc
import concourse.tile as tile
from concourse.bass_utils import run_bass_kernel_spmd
from concourse import mybir

B, L, D, H = 2, 2048, 1024, 16
DH = 64          # dim per head
HPC = 4          # heads per core
CPC = HPC * DH   # feature cols per core = 256
NCORES = 8

MM_DT = "bfloat16"
NP_MM = ml_dtypes.bfloat16 if MM_DT == "bfloat16" else np.float32

_CACHE = {}


def build_nc(mm_dt: str):
    nc = bacc.Bacc()
    mm_dt = mybir.dt(mm_dt)
    fp32 = mybir.dt.float32

    # x tensors pre-transposed on host to [chunk, p, dc, c] so every DMA
    # descriptor is an 8KB contiguous run per partition
    xq = nc.declare_dram_parameter("xq", (4, 128, 8, 512), mm_dt, isOutput=False)
    xk = nc.declare_dram_parameter("xk", (4, 128, 8, 512), mm_dt, isOutput=False)
    xv = nc.declare_dram_parameter("xv", (4, 128, 8, 512), mm_dt, isOutput=False)
    # weights pre-transposed to [p, dc, c]
    wq = nc.declare_dram_parameter("wq", (128, 8, CPC), mm_dt, isOutput=False)
    wk = nc.declare_dram_parameter("wk", (128, 8, CPC), mm_dt, isOutput=False)
    wv = nc.declare_dram_parameter("wv", (128, 8, CPC), mm_dt, isOutput=False)
    wo = nc.declare_dram_parameter("wo", (CPC, D), mm_dt, isOutput=False)
    bq = nc.declare_dram_parameter("bq", (CPC, 1), fp32, isOutput=False)
    bk = nc.declare_dram_parameter("bk", (CPC, 1), fp32, isOutput=False)
    y = nc.declare_dram_parameter("y", (L, D), mm_dt, isOutput=True)      # partial out

    from contextlib import ExitStack

    with ExitStack() as es:
        tc = es.enter_context(tile.TileContext(nc))
        # NOTE: bufs are per named tag
        warm_pool = es.enter_context(tc.tile_pool(name="warm", bufs=1))
        xt_pool = es.enter_context(tc.tile_pool(name="xt", bufs=1))     # 3 tags [128,4,8,512]
        w_pool = es.enter_context(tc.tile_pool(name="w", bufs=1))       # 3 tags [128,8,256]
        wo_pool = es.enter_context(tc.tile_pool(name="wo", bufs=1))     # 2 tags [128,1024]
        bias_pool = es.enter_context(tc.tile_pool(name="bias", bufs=1))
        qt_pool = es.enter_context(tc.tile_pool(name="qt", bufs=1))     # 2 tags [128,2048]
        kt_pool = es.enter_context(tc.tile_pool(name="kt", bufs=1))
        vn_pool = es.enter_context(tc.tile_pool(name="vn", bufs=1))     # [128,16,4,65]
        pt_pool = es.enter_context(tc.tile_pool(name="pt", bufs=5))     # [128,1024]
        zr_pool = es.enter_context(tc.tile_pool(name="zr", bufs=3))     # [1,512]
        zbs_pool = es.enter_context(tc.tile_pool(name="zbs", bufs=3))   # [64,512]
        ot_pool = es.enter_context(tc.tile_pool(name="ot", bufs=1))     # 2 tags [128,2048]
        y_pool = es.enter_context(tc.tile_pool(name="ysb", bufs=3))     # [128,1024]
        psA = es.enter_context(tc.tile_pool(name="psA", bufs=2, space="PSUM"))
        psS = es.enter_context(tc.tile_pool(name="psS", bufs=2, space="PSUM"))
        psOT = es.enter_context(tc.tile_pool(name="psOT", bufs=2, space="PSUM"))
        if True:
            # ---- engine warm-up during the DMA lead-in ----------------------
            # GpSimd partition_broadcast lives in a different microcode
            # library than affine_select; its first use triggers a ~10us
            # LIBRARY_RELOAD. Touch every library (and the ACT exp table)
            # while the input DMAs stream in, off the critical path.
            wz = warm_pool.tile([64, 8], fp32, name="wz")
            nc.vector.memset(wz, 1.0)
            wz2 = warm_pool.tile([64, 8], fp32, name="wz2")
            nc.gpsimd.partition_broadcast(out_ap=wz2, in_ap=wz[0:1, :])
            nc.gpsimd.affine_select(
                out=wz2, in_=wz2,
                compare_op=mybir.AluOpType.is_ge,
                fill=0.0, base=0, channel_multiplier=-1, pattern=[[1, 8]],
            )
            wz3 = warm_pool.tile([64, 8], mm_dt, name="wz3")
            nc.scalar.activation(
                out=wz3, in_=wz,
                func=mybir.ActivationFunctionType.Exp, scale=0.125,
            )
            # PE clock pre-ramp: the PE only reaches max frequency after ~3us
            # of continuous execution. Stream junk matmuls through it during
            # the DMA lead-in so the real prologue starts at full clock.
            dm_w = warm_pool.tile([128, 128], mm_dt, name="dmw")
            dm_x = warm_pool.tile([128, 512], mm_dt, name="dmx")
            nc.vector.memset(dm_w, 0.0)
            nc.vector.memset(dm_x, 0.0)
            dps = psS.tile([128, 1024], fp32, name="st2")
            for _ in range(30):
                nc.tensor.matmul(dps[:, 0:512], dm_w, dm_x, start=True, stop=True)

            # ---- load inputs (DMA queue order == consumption order) ---------
            wk_sb = w_pool.tile([128, 8, CPC], mm_dt, name="wk")
            nc.sync.dma_start(out=wk_sb, in_=wk.rearrange("p dc c -> p dc c"))
            xk_sb = xt_pool.tile([128, 4, 8, 512], mm_dt, name="xk")
            xk_r = xk.rearrange("ch p dc c -> p ch dc c")
            nc.sync.dma_start(out=xk_sb[:, 0], in_=xk_r[:, 0])
            wq_sb = w_pool.tile([128, 8, CPC], mm_dt, name="wq")
            nc.sync.dma_start(out=wq_sb, in_=wq.rearrange("p dc c -> p dc c"))
            xq_sb = xt_pool.tile([128, 4, 8, 512], mm_dt, name="xq")
            xq_r = xq.rearrange("ch p dc c -> p ch dc c")
            nc.sync.dma_start(out=xq_sb[:, 0], in_=xq_r[:, 0])
            bk_sb = bias_pool.tile([128, 2], fp32, name="bk")
            nc.sync.dma_start(out=bk_sb, in_=bk.rearrange("(cc p) o -> p cc o", p=128))
            bq_sb = bias_pool.tile([128, 2], fp32, name="bq")
            nc.sync.dma_start(out=bq_sb, in_=bq.rearrange("(cc p) o -> p cc o", p=128))

            wv_sb = w_pool.tile([128, 8, CPC], mm_dt, name="wv")
            nc.sync.dma_start(out=wv_sb, in_=wv.rearrange("p dc c -> p dc c"))
            xv_sb = xt_pool.tile([128, 4, 8, 512], mm_dt, name="xv")
            xv_r = xv.rearrange("ch p dc c -> p ch dc c")
            nc.sync.dma_start(out=xv_sb[:, 0], in_=xv_r[:, 0])

            for ch in range(1, 4):
                nc.sync.dma_start(out=xk_sb[:, ch], in_=xk_r[:, ch])
                nc.sync.dma_start(out=xq_sb[:, ch], in_=xq_r[:, ch])
                nc.sync.dma_start(out=xv_sb[:, ch], in_=xv_r[:, ch])

            wo_sb = []
            for cc in range(2):
                t = wo_pool.tile([128, D], mm_dt, name=f"wo{cc}")
                nc.sync.dma_start(out=t, in_=wo[cc * 128:(cc + 1) * 128, :])
                wo_sb.append(t)

            # ---- stage A helpers --------------------------------------------
            qt_sb = [qt_pool.tile([128, L], mm_dt, name=f"qt{i}") for i in range(2)]
            kt_sb = [kt_pool.tile([128, L], mm_dt, name=f"kt{i}") for i in range(2)]
            # V natural layout: [128(lt-part), 16 lt, 4 head, 65] (col 64 = ones)
            v_sb = vn_pool.tile([128, 16, 4, 65], mm_dt)
            nc.vector.memset(v_sb[:, :, :, 64:65], 1.0)

            def emit_QK_cc(dst, x_sb, w_sb, b_sb, lg, cc, nsub=1,
                           part=None, ps_store=None):
                # nsub>1: accumulate in column sub-chunks so the first matmul
                # only depends on a partial input DMA (shrinks lead-in).
                # part=0/1 with ps_store: emit only dc 0-3 / 4-7 (finer filler
                # quanta), sharing the psum tile via ps_store[key]
                if part == 1:
                    ps = ps_store.pop(('qk', id(dst), lg, cc))
                    dcs = range(4, 8)
                else:
                    ps = psA.tile([128, 512], fp32)
                    if part == 0:
                        ps_store[('qk', id(dst), lg, cc)] = ps
                        dcs = range(0, 4)
                    else:
                        dcs = range(0, 8)
                sw = 512 // nsub
                for sub in range(nsub):
                    s0 = sub * sw
                    for dc in dcs:
                        nc.tensor.matmul(
                            ps[:, s0:s0 + sw],
                            w_sb[:, dc, cc * 128:(cc + 1) * 128],
                            x_sb[:, lg, dc, s0:s0 + sw],
                            start=(dc == 0),
                            stop=(dc == 7),
                        )
                if part != 0:
                    nc.vector.tensor_scalar_add(
                        out=dst[cc][:, lg * 512:(lg + 1) * 512],
                        in0=ps,
                        scalar1=b_sb[:, cc:cc + 1],
                    )

            def emit_V(lt):
                ps = psA.tile([128, CPC], fp32)
                for dc in range(8):
                    nc.tensor.matmul(
                        ps,
                        xv_sb[:, lt // 4, dc, (lt % 4) * 128:(lt % 4) * 128 + 128],
                        wv_sb[:, dc, :],
                        start=(dc == 0),
                        stop=(dc == 7),
                    )
                if lt < 8:
                    # early groups: DVE is the congested engine, ACT idles
                    nc.scalar.activation(
                        out=v_sb[:, lt, :, 0:64],
                        in_=ps.rearrange("p (h d) -> p h d", d=64),
                        func=mybir.ActivationFunctionType.Copy,
                        bias=0.0,
                    )
                else:
                    nc.vector.tensor_copy(
                        out=v_sb[:, lt, :, 0:64],
                        in_=ps.rearrange("p (h d) -> p h d", d=64),
                    )

            ot_sb = [ot_pool.tile([128, L], mm_dt, name=f"ot{i}") for i in range(2)]
            y_view = y.rearrange("(lt p) c -> p lt c", p=128)

            cur_y = {}

            def emit_C_piece(g4, li, dg, use_act=False):
                # one (128 q-rows, 512 out-cols) piece of the output
                # projection; an lt's two dg pieces share one [128,1024] SBUF
                # tile and go out as a single contiguous DMA (halves the
                # output descriptor count -- the final descriptor-ring
                # refills gate the kernel end)
                lt = g4 * 4 + li
                ps = psA.tile([128, 512], fp32)
                for cc in range(2):
                    nc.tensor.matmul(
                        ps,
                        ot_sb[cc][:, lt * 128:(lt + 1) * 128],
                        wo_sb[cc][:, dg * 512:(dg + 1) * 512],
                        start=(cc == 0),
                        stop=(cc == 1),
                    )
                if dg == 0:
                    cur_y[lt] = y_pool.tile([128, 1024], mm_dt, name='yt')
                yt = cur_y[lt]
                if use_act:
                    nc.scalar.activation(
                        out=yt[:, dg * 512:(dg + 1) * 512], in_=ps,
                        func=mybir.ActivationFunctionType.Copy,
                        bias=0.0,
                    )
                else:
                    nc.vector.tensor_copy(
                        out=yt[:, dg * 512:(dg + 1) * 512], in_=ps)
                if dg == 1:
                    nc.sync.dma_start(
                        out=y_view[:, lt, :],
                        in_=cur_y.pop(lt),
                    )

            # ---- filler queue: small PE work quanta pumped into the S/P
            # stream wherever the exp pipeline (ACT) runs behind the PE ------
            QK_NS, V_NS, C_NS = 1720.0, 864.0, 430.0
            fillers = []   # (cost_ns, tag, emit_fn)  tag: ('V', lt)|('QK', g)|('C',)
            reserve = []   # C pieces held back to cover the final divide
            ps_store = {}
            state = {"deficit": 0.0}

            def _emit_head():
                cost, _tag, fn = fillers.pop(0)
                fn()
                state["deficit"] -= cost

            def pump():
                while fillers and state["deficit"] >= 0.5 * fillers[0][0]:
                    _emit_head()

            def force_V(max_kt):
                # everything queued up to and including ('V', max_kt) must be
                # emitted before the P matmul that consumes v_sb[:, max_kt]
                while fillers:
                    need = any(
                        t[0] == 'V' and t[1] <= max_kt
                        for _, t, _ in fillers
                    )
                    if not need:
                        break
                    _emit_head()

            def force_QK(g4):
                while fillers:
                    need = any(
                        t[0] == 'QK' and t[1] <= g4
                        for _, t, _ in fillers
                    )
                    if not need:
                        break
                    _emit_head()

            def _add_qk(dst, x_sb, w_sb, b_sb, g, cc):
                for part in (0, 1):
                    fillers.append((QK_NS / 2, ('QK', g), (
                        lambda part=part: emit_QK_cc(
                            dst, x_sb, w_sb, b_sb, g, cc,
                            part=part, ps_store=ps_store))))

            for lt in range(4):
                fillers.append((V_NS, ('V', lt), (lambda lt=lt: emit_V(lt))))
            for g in range(1, 4):
                for cc in range(2):
                    _add_qk(kt_sb, xk_sb, wk_sb, bk_sb, g, cc)
                for cc in range(2):
                    _add_qk(qt_sb, xq_sb, wq_sb, bq_sb, g, cc)
                if g < 3:
                    for lt in range(4 * g, 4 * g + 4):
                        fillers.append((V_NS, ('V', lt), (lambda lt=lt: emit_V(lt))))
                else:
                    # V(12..15) deferred into g4=3's stream (forced by its Ps)
                    pass
                if g == 2:
                    for (li, dg) in [(0, 0), (0, 1), (1, 0), (1, 1)]:
                        fillers.append((C_NS, ('C',), (
                            lambda li=li, dg=dg: emit_C_piece(0, li, dg))))
            for (li, dg) in [(2, 0), (2, 1), (3, 0), (3, 1)]:
                fillers.append((C_NS, ('C',), (
                    lambda li=li, dg=dg: emit_C_piece(0, li, dg))))
            for lt in range(12, 16):
                fillers.append((V_NS, ('V', lt), (lambda lt=lt: emit_V(lt))))
            for (li, dg) in [(li, dg) for li in range(4) for dg in range(2)]:
                fillers.append((C_NS, ('C',), (
                    lambda li=li, dg=dg: emit_C_piece(1, li, dg))))
            for i, (li, dg) in enumerate(
                    [(li, dg) for li in range(4) for dg in range(2)]):
                if i < 2:
                    fillers.append((C_NS, ('C',), (
                        lambda li=li, dg=dg: emit_C_piece(2, li, dg))))
                else:
                    # held back: these cover the PE through the final divide
                    reserve.append((li, dg))

            # prologue: Q/K projections for the first 512 queries
            emit_QK_cc(kt_sb, xk_sb, wk_sb, bk_sb, 0, 0)
            emit_QK_cc(kt_sb, xk_sb, wk_sb, bk_sb, 0, 1)
            emit_QK_cc(qt_sb, xq_sb, wq_sb, bq_sb, 0, 0)
            emit_QK_cc(qt_sb, xq_sb, wq_sb, bq_sb, 0, 1)

            # ---- stage B + C interleaved ------------------------------------
            for g4 in range(4):
                force_QK(g4)
                for h in range(HPC):
                    cc = h // 2
                    ro = (h % 2) * 64
                    nkt = g4 * 4 + 4
                    ot_ps = psOT.tile([65, 512], fp32)
                    pts = {}

                    def emit_S_pair(k0):
                        # two kt tiles share a [128,1024] PSUM pair; a single
                        # wide exp covers the pair (ACT per-instr overhead is
                        # large). For diag pairs the exp starts at col off0
                        # (everything left of it is fully masked anyway) and
                        # affine_select zeroes the masked staircase after.
                        diag = (k0 // 4 == g4)
                        st = psS.tile([128, 1024], fp32, name="st2")
                        o0 = 128 * (k0 % 4) if diag else 0
                        for j in range(2):
                            kt = k0 + j
                            off = 128 * (kt % 4) if diag else 0
                            base = j * 512
                            nc.tensor.matmul(
                                st[:, base + off:base + 512],
                                kt_sb[cc][ro:ro + 64, kt * 128:(kt + 1) * 128],
                                qt_sb[cc][ro:ro + 64,
                                          g4 * 512 + off:(g4 + 1) * 512],
                                start=True,
                                stop=True,
                            )
                        pt = pt_pool.tile([128, 1024], mm_dt, name="pt2")
                        if diag and o0 >= 256:
                            # two narrow exps skip the fully-masked middle
                            nc.scalar.activation(
                                out=pt[:, o0:512], in_=st[:, o0:512],
                                func=mybir.ActivationFunctionType.Exp,
                                scale=0.125,
                            )
                            nc.scalar.activation(
                                out=pt[:, 512 + o0 + 128:1024],
                                in_=st[:, 512 + o0 + 128:1024],
                                func=mybir.ActivationFunctionType.Exp,
                                scale=0.125,
                            )
                        else:
                            nc.scalar.activation(
                                out=pt[:, o0:1024],
                                in_=st[:, o0:1024],
                                func=mybir.ActivationFunctionType.Exp,
                                scale=0.125,
                            )
                        if diag:
                            for j in range(2):
                                kt = k0 + j
                                off = 128 * (kt % 4)
                                base = j * 512
                                # keep iff f - p - off >= 0. Cols >= off+128
                                # all-keep (skip); cols < off all-fill (zeroes
                                # the masked/stale region the exp skipped or
                                # saw as garbage).
                                w = off + 128
                                nc.gpsimd.affine_select(
                                    out=pt[:, base:base + w],
                                    in_=pt[:, base:base + w],
                                    compare_op=mybir.AluOpType.is_ge,
                                    fill=0.0,
                                    base=-off,
                                    channel_multiplier=-1,
                                    pattern=[[1, w]],
                                )
                            s_ns = 0.42 * (896.0 - 2 * o0)
                            if o0 >= 256:
                                e_ns = 0.87 * (896.0 - 2 * o0) + 460.0
                            else:
                                e_ns = 0.87 * (1024.0 - o0) + 230.0
                        else:
                            s_ns = 430.0
                            e_ns = 1113.0
                        state["deficit"] = min(
                            2500.0, state["deficit"] + e_ns - s_ns)
                        pts[k0] = pt[:, 0:512]
                        pts[k0 + 1] = pt[:, 512:1024]

                    def emit_P(kt):
                        # diag tiles: pt cols [0, off) are fully-masked zeros
                        # -- skip streaming them (their contribution is 0)
                        off = 128 * (kt % 4) if (kt // 4 == g4) else 0
                        nc.tensor.matmul(
                            ot_ps[:, off:512],
                            v_sb[:, kt, h, :],
                            pts.pop(kt)[:, off:512],
                            start=(kt == 0),
                            stop=(kt == nkt - 1),
                        )
                        return 512 - off

                    def emit_P_pair(k0):
                        force_V(k0 + 1)
                        n = emit_P(k0)
                        n += emit_P(k0 + 1)
                        state["deficit"] = max(
                            -4000.0, state["deficit"] - 0.42 * n)

                    npair = nkt // 2
                    for kp in range(npair):
                        if kp >= 1:
                            emit_P_pair(2 * kp - 2)
                            pump()
                        emit_S_pair(2 * kp)
                        pump()
                    emit_P_pair(nkt - 2)
                    pump()

                    # divide by Z (row 64) -- off the PE entirely.
                    # NB: reciprocal_approx_fast reading PSUM directly is
                    # silently wrong; bounce the row through SBUF first.
                    last = (g4 == 3 and h == 3)
                    if last:
                        # drain remaining quanta now: they keep the PE busy
                        # through the final divide chain instead of landing
                        # after the C(3) pieces in the tail
                        while fillers:
                            _emit_head()
                        for i, (li, dg) in enumerate(reserve):
                            emit_C_piece(2, li, dg, use_act=(i % 2 == 0))
                    nsp = 4 if last else 1
                    hw_ = 512 // nsp
                    for sp in range(nsp):
                        s0 = sp * hw_
                        zrow = zr_pool.tile([1, 512], fp32, name="zrow")
                        if g4 <= 1:
                            nc.scalar.activation(
                                out=zrow[:, s0:s0 + hw_],
                                in_=ot_ps[64:65, s0:s0 + hw_],
                                func=mybir.ActivationFunctionType.Copy,
                                bias=0.0,
                            )
                        else:
                            nc.vector.tensor_copy(
                                out=zrow[:, s0:s0 + hw_],
                                in_=ot_ps[64:65, s0:s0 + hw_])
                        zr = zr_pool.tile([1, 512], fp32, name="zr")
                        nc.vector.reciprocal_approx_fast(
                            out=zr[:, s0:s0 + hw_], in_=zrow[:, s0:s0 + hw_])
                        zb = zbs_pool.tile([64, 512], fp32)
                        nc.gpsimd.partition_broadcast(
                            out_ap=zb[:, s0:s0 + hw_], in_ap=zr[:, s0:s0 + hw_])
                        nc.vector.tensor_mul(
                            out=ot_sb[cc][ro:ro + 64,
                                          g4 * 512 + s0:g4 * 512 + s0 + hw_],
                            in0=ot_ps[0:64, s0:s0 + hw_],
                            in1=zb[:, s0:s0 + hw_],
                        )
                        if last:
                            # final output pieces pipeline behind each quarter
                            for dg in range(2):
                                emit_C_piece(3, sp, dg, use_act=(dg == 1))

    nc.compile()
    return nc


def _get_nc(mm_dt: str):
    if mm_dt not in _CACHE:
        _CACHE[mm_dt] = build_nc(mm_dt)
    return _CACHE[mm_dt]


def _x_host(a):
    # [D, L] -> [chunk, p, dc, c]: value (ch, p, dc, c) = a[dc*128+p, ch*512+c]
    return np.ascontiguousarray(
        a.reshape(8, 128, 4, 512).transpose(2, 1, 0, 3)).astype(NP_MM)


def _w_host(w):
    # [D, CPC] -> [p, dc, c]
    return np.ascontiguousarray(
        w.reshape(8, 128, CPC).transpose(1, 0, 2)).astype(NP_MM)


def kernel(q, k, v, mask, Wq, bq, Wk, bk, Wv, bv, Wo, bo, _trace=False):
    nc = _get_nc(MM_DT)

    in_maps = []
    for c in range(NCORES):
        b = c // 4
        g = c % 4
        s = slice(g * CPC, (g + 1) * CPC)
        in_maps.append({
            "xq": _x_host(q[b].T),
            "xk": _x_host(k[b].T),
            "xv": _x_host(v[b].T),
            "wq": _w_host(Wq[:, s]),
            "wk": _w_host(Wk[:, s]),
            "wv": _w_host(Wv[:, s]),
            "wo": np.ascontiguousarray(Wo[s, :]).astype(NP_MM),
            "bq": np.ascontiguousarray(bq[s]).reshape(CPC, 1).astype(np.float32),
            "bk": np.ascontiguousarray(bk[s]).reshape(CPC, 1).astype(np.float32),
        })

    res = run_bass_kernel_spmd(nc, in_maps, list(range(NCORES)), trace=_trace)

    # host gather: out[b] = sum_g y_core(b,g) + (bo + bv @ Wo)
    const = (bo + bv.astype(np.float64) @ Wo.astype(np.float64)).astype(np.float64)
    out = np.zeros((B, L, D), np.float64)
    for c in range(NCORES):
        out[c // 4] += res.results[c]["y"].astype(np.float64)
    out += const[None, None, :]
    kernel.last_exec_time_ns = res.exec_time_ns
    return out.astype(np.float32)


# revision 29
# speedup vs baseline: 1.0049x; 1.0049x over previous
import sys

sys.path.insert(0, "/opt/trn_rl_repo")

import numpy as np
import ml_dtypes

import concourse.bass as bass
import concourse.bacc as bacc
import concourse.tile as tile
from concourse.bass_utils import run_bass_kernel_spmd
from concourse import mybir

B, L, D, H = 2, 2048, 1024, 16
DH = 64          # dim per head
HPC = 4          # heads per core
CPC = HPC * DH   # feature cols per core = 256
NCORES = 8

MM_DT = "bfloat16"
NP_MM = ml_dtypes.bfloat16 if MM_DT == "bfloat16" else np.float32

_CACHE = {}


def build_nc(mm_dt: str):
    nc = bacc.Bacc()
    mm_dt = mybir.dt(mm_dt)
    fp32 = mybir.dt.float32

    # x tensors pre-transposed on host to [chunk, p, dc, c] so every DMA
    # descriptor is an 8KB contiguous run per partition
    xq = nc.declare_dram_parameter("xq", (4, 128, 8, 512), mm_dt, isOutput=False)
    xk = nc.declare_dram_parameter("xk", (4, 128, 8, 512), mm_dt, isOutput=False)
    xv = nc.declare_dram_parameter("xv", (4, 128, 8, 512), mm_dt, isOutput=False)
    # weights pre-transposed to [p, dc, c]
    wq = nc.declare_dram_parameter("wq", (128, 8, CPC), mm_dt, isOutput=False)
    wk = nc.declare_dram_parameter("wk", (128, 8, CPC), mm_dt, isOutput=False)
    wv = nc.declare_dram_parameter("wv", (128, 8, CPC), mm_dt, isOutput=False)
    wo = nc.declare_dram_parameter("wo", (CPC, D), mm_dt, isOutput=False)
    bq = nc.declare_dram_parameter("bq", (CPC, 1), fp32, isOutput=False)
    bk = nc.declare_dram_parameter("bk", (CPC, 1), fp32, isOutput=False)
    y = nc.declare_dram_parameter("y", (L, D), mm_dt, isOutput=True)      # partial out

    from contextlib import ExitStack

    with ExitStack() as es:
        tc = es.enter_context(tile.TileContext(nc))
        # NOTE: bufs are per named tag
        warm_pool = es.enter_context(tc.tile_pool(name="warm", bufs=1))
        xt_pool = es.enter_context(tc.tile_pool(name="xt", bufs=1))     # 3 tags [128,4,8,512]
        w_pool = es.enter_context(tc.tile_pool(name="w", bufs=1))       # 3 tags [128,8,256]
        wo_pool = es.enter_context(tc.tile_pool(name="wo", bufs=1))     # 2 tags [128,1024]
        bias_pool = es.enter_context(tc.tile_pool(name="bias", bufs=1))
        qt_pool = es.enter_context(tc.tile_pool(name="qt", bufs=1))     # 2 tags [128,2048]
        kt_pool = es.enter_context(tc.tile_pool(name="kt", bufs=1))
        vn_pool = es.enter_context(tc.tile_pool(name="vn", bufs=1))     # [128,16,4,65]
        pt_pool = es.enter_context(tc.tile_pool(name="pt", bufs=5))     # [128,1024]
        zr_pool = es.enter_context(tc.tile_pool(name="zr", bufs=3))     # [1,512]
        zbs_pool = es.enter_context(tc.tile_pool(name="zbs", bufs=3))   # [64,512]
        ot_pool = es.enter_context(tc.tile_pool(name="ot", bufs=1))     # 2 tags [128,2048]
        y_pool = es.enter_context(tc.tile_pool(name="ysb", bufs=3))     # [128,1024]
        psA = es.enter_context(tc.tile_pool(name="psA", bufs=2, space="PSUM"))
        psS = es.enter_context(tc.tile_pool(name="psS", bufs=2, space="PSUM"))
        psOT = es.enter_context(tc.tile_pool(name="psOT", bufs=2, space="PSUM"))
        if True:
            # ---- engine warm-up during the DMA lead-in ----------------------
            # GpSimd partition_broadcast lives in a different microcode
            # library than affine_select; its first use triggers a ~10us
            # LIBRARY_RELOAD. Touch every library (and the ACT exp table)
            # while the input DMAs stream in, off the critical path.
            wz = warm_pool.tile([64, 8], fp32, name="wz")
            nc.vector.memset(wz, 1.0)
            wz2 = warm_pool.tile([64, 8], fp32, name="wz2")
            nc.gpsimd.partition_broadcast(out_ap=wz2, in_ap=wz[0:1, :])
            nc.gpsimd.affine_select(
                out=wz2, in_=wz2,
                compare_op=mybir.AluOpType.is_ge,
                fill=0.0, base=0, channel_multiplier=-1, pattern=[[1, 8]],
            )
            wz3 = warm_pool.tile([64, 8], mm_dt, name="wz3")
            nc.scalar.activation(
                out=wz3, in_=wz,
                func=mybir.ActivationFunctionType.Exp, scale=0.125,
            )
            # PE clock pre-ramp: the PE only reaches max frequency after ~3us
            # of continuous execution. Stream junk matmuls through it during
            # the DMA lead-in so the real prologue starts at full clock.
            dm_w = warm_pool.tile([128, 128], mm_dt, name="dmw")
            dm_x = warm_pool.tile([128, 512], mm_dt, name="dmx")
            nc.vector.memset(dm_w, 0.0)
            nc.vector.memset(dm_x, 0.0)
            dps = psS.tile([128, 1024], fp32, name="st2")
            for _ in range(30):
                nc.tensor.matmul(dps[:, 0:512], dm_w, dm_x, start=True, stop=True)

            # ---- load inputs (DMA queue order == consumption order) ---------
            wk_sb = w_pool.tile([128, 8, CPC], mm_dt, name="wk")
            nc.sync.dma_start(out=wk_sb, in_=wk.rearrange("p dc c -> p dc c"))
            xk_sb = xt_pool.tile([128, 4, 8, 512], mm_dt, name="xk")
            xk_r = xk.rearrange("ch p dc c -> p ch dc c")
            nc.sync.dma_start(out=xk_sb[:, 0], in_=xk_r[:, 0])
            wq_sb = w_pool.tile([128, 8, CPC], mm_dt, name="wq")
            nc.sync.dma_start(out=wq_sb, in_=wq.rearrange("p dc c -> p dc c"))
            xq_sb = xt_pool.tile([128, 4, 8, 512], mm_dt, name="xq")
            xq_r = xq.rearrange("ch p dc c -> p ch dc c")
            nc.sync.dma_start(out=xq_sb[:, 0], in_=xq_r[:, 0])
            bk_sb = bias_pool.tile([128, 2], fp32, name="bk")
            nc.sync.dma_start(out=bk_sb, in_=bk.rearrange("(cc p) o -> p cc o", p=128))
            bq_sb = bias_pool.tile([128, 2], fp32, name="bq")
            nc.sync.dma_start(out=bq_sb, in_=bq.rearrange("(cc p) o -> p cc o", p=128))

            wv_sb = w_pool.tile([128, 8, CPC], mm_dt, name="wv")
            nc.sync.dma_start(out=wv_sb, in_=wv.rearrange("p dc c -> p dc c"))
            xv_sb = xt_pool.tile([128, 4, 8, 512], mm_dt, name="xv")
            xv_r = xv.rearrange("ch p dc c -> p ch dc c")
            nc.sync.dma_start(out=xv_sb[:, 0], in_=xv_r[:, 0])

            for ch in range(1, 4):
                nc.sync.dma_start(out=xk_sb[:, ch], in_=xk_r[:, ch])
                nc.sync.dma_start(out=xq_sb[:, ch], in_=xq_r[:, ch])
                nc.sync.dma_start(out=xv_sb[:, ch], in_=xv_r[:, ch])

            wo_sb = []
            for cc in range(2):
                t = wo_pool.tile([128, D], mm_dt, name=f"wo{cc}")
                nc.sync.dma_start(out=t, in_=wo[cc * 128:(cc + 1) * 128, :])
                wo_sb.append(t)

            # ---- stage A helpers --------------------------------------------
            qt_sb = [qt_pool.tile([128, L], mm_dt, name=f"qt{i}") for i in range(2)]
            kt_sb = [kt_pool.tile([128, L], mm_dt, name=f"kt{i}") for i in range(2)]
            # V natural layout: [128(lt-part), 16 lt, 4 head, 65] (col 64 = ones)
            v_sb = vn_pool.tile([128, 16, 4, 65], mm_dt)
            nc.vector.memset(v_sb[:, :, :, 64:65], 1.0)

            def emit_QK_cc(dst, x_sb, w_sb, b_sb, lg, cc, nsub=1,
                           part=None, ps_store=None):
                # nsub>1: accumulate in column sub-chunks so the first matmul
                # only depends on a partial input DMA (shrinks lead-in).
                # part=0/1 with ps_store: emit only dc 0-3 / 4-7 (finer filler
                # quanta), sharing the psum tile via ps_store[key]
                if part == 1:
                    ps = ps_store.pop(('qk', id(dst), lg, cc))
                    dcs = range(4, 8)
                else:
                    ps = psA.tile([128, 512], fp32)
                    if part == 0:
                        ps_store[('qk', id(dst), lg, cc)] = ps
                        dcs = range(0, 4)
                    else:
                        dcs = range(0, 8)
                sw = 512 // nsub
                for sub in range(nsub):
                    s0 = sub * sw
                    for dc in dcs:
                        nc.tensor.matmul(
                            ps[:, s0:s0 + sw],
                            w_sb[:, dc, cc * 128:(cc + 1) * 128],
                            x_sb[:, lg, dc, s0:s0 + sw],
                            start=(dc == 0),
                            stop=(dc == 7),
                        )
                if part != 0:
                    nc.vector.tensor_scalar_add(
                        out=dst[cc][:, lg * 512:(lg + 1) * 512],
                        in0=ps,
                        scalar1=b_sb[:, cc:cc + 1],
                    )

            def emit_V(lt):
                ps = psA.tile([128, CPC], fp32)
                for dc in range(8):
                    nc.tensor.matmul(
                        ps,
                        xv_sb[:, lt // 4, dc, (lt % 4) * 128:(lt % 4) * 128 + 128],
                        wv_sb[:, dc, :],
                        start=(dc == 0),
                        stop=(dc == 7),
                    )
                if lt < 8:
                    # early groups: DVE is the congested engine, ACT idles
                    nc.scalar.activation(
                        out=v_sb[:, lt, :, 0:64],
                        in_=ps.rearrange("p (h d) -> p h d", d=64),
                        func=mybir.ActivationFunctionType.Copy,
                        bias=0.0,
                    )
                else:
                    nc.vector.tensor_copy(
                        out=v_sb[:, lt, :, 0:64],
                        in_=ps.rearrange("p (h d) -> p h d", d=64),
                    )

            ot_sb = [ot_pool.tile([128, L], mm_dt, name=f"ot{i}") for i in range(2)]
            y_view = y.rearrange("(lt p) c -> p lt c", p=128)

            cur_y = {}

            def emit_C_piece(g4, li, dg, use_act=False):
                # one (128 q-rows, 512 out-cols) piece of the output
                # projection; an lt's two dg pieces share one [128,1024] SBUF
                # tile and go out as a single contiguous DMA (halves the
                # output descriptor count -- the final descriptor-ring
                # refills gate the kernel end)
                lt = g4 * 4 + li
                ps = psA.tile([128, 512], fp32)
                for cc in range(2):
                    nc.tensor.matmul(
                        ps,
                        ot_sb[cc][:, lt * 128:(lt + 1) * 128],
                        wo_sb[cc][:, dg * 512:(dg + 1) * 512],
                        start=(cc == 0),
                        stop=(cc == 1),
                    )
                if dg == 0:
                    cur_y[lt] = y_pool.tile([128, 1024], mm_dt, name='yt')
                yt = cur_y[lt]
                if use_act:
                    nc.scalar.activation(
                        out=yt[:, dg * 512:(dg + 1) * 512], in_=ps,
                        func=mybir.ActivationFunctionType.Copy,
                        bias=0.0,
                    )
                else:
                    nc.vector.tensor_copy(
                        out=yt[:, dg * 512:(dg + 1) * 512], in_=ps)
                if dg == 1:
                    nc.sync.dma_start(
                        out=y_view[:, lt, :],
                        in_=cur_y.pop(lt),
                    )

            # ---- filler queue: small PE work quanta pumped into the S/P
            # stream wherever the exp pipeline (ACT) runs behind the PE ------
            QK_NS, V_NS, C_NS = 1720.0, 864.0, 430.0
            fillers = []   # (cost_ns, tag, emit_fn)  tag: ('V', lt)|('QK', g)|('C',)
            reserve = []   # C pieces held back to cover the final divide
            ps_store = {}
            state = {"deficit": 0.0}

            def _emit_head():
                cost, _tag, fn = fillers.pop(0)
                fn()
                state["deficit"] -= cost

            def pump():
                while fillers and state["deficit"] >= 0.5 * fillers[0][0]:
                    _emit_head()

            def force_V(max_kt):
                # everything queued up to and including ('V', max_kt) must be
                # emitted before the P matmul that consumes v_sb[:, max_kt]
                while fillers:
                    need = any(
                        t[0] == 'V' and t[1] <= max_kt
                        for _, t, _ in fillers
                    )
                    if not need:
                        break
                    _emit_head()

            def force_QK(g4):
                while fillers:
                    need = any(
                        t[0] == 'QK' and t[1] <= g4
                        for _, t, _ in fillers
                    )
                    if not need:
                        break
                    _emit_head()

            def _add_qk(dst, x_sb, w_sb, b_sb, g, cc):
                for part in (0, 1):
                    fillers.append((QK_NS / 2, ('QK', g), (
                        lambda part=part: emit_QK_cc(
                            dst, x_sb, w_sb, b_sb, g, cc,
                            part=part, ps_store=ps_store))))

            for lt in range(4):
                fillers.append((V_NS, ('V', lt), (lambda lt=lt: emit_V(lt))))
            for g in range(1, 4):
                for cc in range(2):
                    _add_qk(kt_sb, xk_sb, wk_sb, bk_sb, g, cc)
                for cc in range(2):
                    _add_qk(qt_sb, xq_sb, wq_sb, bq_sb, g, cc)
                if g < 3:
                    for lt in range(4 * g, 4 * g + 4):
                        fillers.append((V_NS, ('V', lt), (lambda lt=lt: emit_V(lt))))
                else:
                    # V(12..15) deferred into g4=3's stream (forced by its Ps)
                    pass
                if g == 2:
                    for (li, dg) in [(0, 0), (0, 1), (1, 0), (1, 1)]:
                        fillers.append((C_NS, ('C',), (
                            lambda li=li, dg=dg: emit_C_piece(0, li, dg))))
            for (li, dg) in [(2, 0), (2, 1), (3, 0), (3, 1)]:
                fillers.append((C_NS, ('C',), (
                    lambda li=li, dg=dg: emit_C_piece(0, li, dg))))
            for lt in range(12, 16):
                fillers.append((V_NS, ('V', lt), (lambda lt=lt: emit_V(lt))))
            for (li, dg) in [(li, dg) for li in range(4) for dg in range(2)]:
                fillers.append((C_NS, ('C',), (
                    lambda li=li, dg=dg: emit_C_piece(1, li, dg))))
            for i, (li, dg) in enumerate(
                    [(li, dg) for li in range(4) for dg in range(2)]):
                if i < 2:
                    fillers.append((C_NS, ('C',), (
                        lambda li=li, dg=dg: emit_C_piece(2, li, dg))))
                else:
                    # held back: these cover the PE through the final divide
                    reserve.append((li, dg))

            # prologue: Q/K projections for the first 512 queries
            emit_QK_cc(kt_sb, xk_sb, wk_sb, bk_sb, 0, 0)
            emit_QK_cc(kt_sb, xk_sb, wk_sb, bk_sb, 0, 1)
            emit_QK_cc(qt_sb, xq_sb, wq_sb, bq_sb, 0, 0)
            emit_QK_cc(qt_sb, xq_sb, wq_sb, bq_sb, 0, 1)

            # ---- stage B + C interleaved ------------------------------------
            for g4 in range(4):
                force_QK(g4)
                for h in range(HPC):
                    cc = h // 2
                    ro = (h % 2) * 64
                    nkt = g4 * 4 + 4
                    ot_ps = psOT.tile([65, 512], fp32)
                    pts = {}

                    def emit_S_pair(k0):
                        # two kt tiles share a [128,1024] PSUM pair; a single
                        # wide exp covers the pair (ACT per-instr overhead is
                        # large). For diag pairs the exp starts at col off0
                        # (everything left of it is fully masked anyway) and
                        # affine_select zeroes the masked staircase after.
                        diag = (k0 // 4 == g4)
                        st = psS.tile([128, 1024], fp32, name="st2")
                        o0 = 128 * (k0 % 4) if diag else 0
                        for j in range(2):
                            kt = k0 + j
                            off = 128 * (kt % 4) if diag else 0
                            base = j * 512
                            nc.tensor.matmul(
                                st[:, base + off:base + 512],
                                kt_sb[cc][ro:ro + 64, kt * 128:(kt + 1) * 128],
                                qt_sb[cc][ro:ro + 64,
                                          g4 * 512 + off:(g4 + 1) * 512],
                                start=True,
                                stop=True,
                            )
                        pt = pt_pool.tile([128, 1024], mm_dt, name="pt2")
                        if diag and o0 >= 256:
                            # two narrow exps skip the fully-masked middle
                            nc.scalar.activation(
                                out=pt[:, o0:512], in_=st[:, o0:512],
                                func=mybir.ActivationFunctionType.Exp,
                                scale=0.125,
                            )
                            nc.scalar.activation(
                                out=pt[:, 512 + o0 + 128:1024],
                                in_=st[:, 512 + o0 + 128:1024],
                                func=mybir.ActivationFunctionType.Exp,
                                scale=0.125,
                            )
                        else:
                            nc.scalar.activation(
                                out=pt[:, o0:1024],
                                in_=st[:, o0:1024],
                                func=mybir.ActivationFunctionType.Exp,
                                scale=0.125,
                            )
                        if diag:
                            for j in range(2):
                                kt = k0 + j
                                off = 128 * (kt % 4)
                                base = j * 512
                                # keep iff f - p - off >= 0. Cols >= off+128
                                # all-keep (skip); cols < off all-fill (zeroes
                                # the masked/stale region the exp skipped or
                                # saw as garbage).
                                w = off + 128
                                nc.gpsimd.affine_select(
                                    out=pt[:, base:base + w],
                                    in_=pt[:, base:base + w],
                                    compare_op=mybir.AluOpType.is_ge,
                                    fill=0.0,
                                    base=-off,
                                    channel_multiplier=-1,
                                    pattern=[[1, w]],
                                )
                            s_ns = 0.42 * (896.0 - 2 * o0)
                            if o0 >= 256:
                                e_ns = 0.87 * (896.0 - 2 * o0) + 460.0
                            else:
                                e_ns = 0.87 * (1024.0 - o0) + 230.0
                        else:
                            s_ns = 430.0
                            e_ns = 1113.0
                        state["deficit"] = min(
                            2500.0, state["deficit"] + e_ns - s_ns)
                        pts[k0] = pt[:, 0:512]
                        pts[k0 + 1] = pt[:, 512:1024]

                    def emit_P(kt):
                        # diag tiles: pt cols [0, off) are fully-masked zeros
                        # -- skip streaming them (their contribution is 0)
                        off = 128 * (kt % 4) if (kt // 4 == g4) else 0
                        nc.tensor.matmul(
                            ot_ps[:, off:512],
                            v_sb[:, kt, h, :],
                            pts.pop(kt)[:, off:512],
                            start=(kt == 0),
                            stop=(kt == nkt - 1),
                        )
                        return 512 - off

                    def emit_P_pair(k0):
                        force_V(k0 + 1)
                        n = emit_P(k0)
                        n += emit_P(k0 + 1)
                        state["deficit"] = max(
                            -4000.0, state["deficit"] - 0.42 * n)

                    npair = nkt // 2
                    for kp in range(npair):
                        emit_S_pair(2 * kp)
                        pump()
                        if kp >= 1:
                            emit_P_pair(2 * kp - 2)
                            pump()
                    emit_P_pair(nkt - 2)
                    pump()

                    # divide by Z (row 64) -- off the PE entirely.
                    # NB: reciprocal_approx_fast reading PSUM directly is
                    # silently wrong; bounce the row through SBUF first.
                    last = (g4 == 3 and h == 3)
                    if last:
                        # drain remaining quanta now: they keep the PE busy
                        # through the final divide chain instead of landing
                        # after the C(3) pieces in the tail
                        while fillers:
                            _emit_head()
                        for (li, dg) in reserve:
                            emit_C_piece(2, li, dg, use_act=True)
                    nsp = 4 if last else 1
                    hw_ = 512 // nsp
                    for sp in range(nsp):
                        s0 = sp * hw_
                        zrow = zr_pool.tile([1, 512], fp32, name="zrow")
                        if g4 <= 1:
                            nc.scalar.activation(
                                out=zrow[:, s0:s0 + hw_],
                                in_=ot_ps[64:65, s0:s0 + hw_],
                                func=mybir.ActivationFunctionType.Copy,
                                bias=0.0,
                            )
                        else:
                            nc.vector.tensor_copy(
                                out=zrow[:, s0:s0 + hw_],
                                in_=ot_ps[64:65, s0:s0 + hw_])
                        zr = zr_pool.tile([1, 512], fp32, name="zr")
                        nc.vector.reciprocal_approx_fast(
                            out=zr[:, s0:s0 + hw_], in_=zrow[:, s0:s0 + hw_])
                        zb = zbs_pool.tile([64, 512], fp32)
                        nc.gpsimd.partition_broadcast(
                            out_ap=zb[:, s0:s0 + hw_], in_ap=zr[:, s0:s0 + hw_])
                        nc.vector.tensor_mul(
                            out=ot_sb[cc][ro:ro + 64,
                                          g4 * 512 + s0:g4 * 512 + s0 + hw_],
                            in0=ot_ps[0:64, s0:s0 + hw_],
                            in1=zb[:, s0:s0 + hw_],
                        )
                        if last:
                            # final output pieces pipeline behind each quarter
                            for dg in range(2):
                                emit_C_piece(3, sp, dg, use_act=(dg == 1))

    nc.compile()
    return nc


def _get_nc(mm_dt: str):
    if mm_dt not in _CACHE:
        _CACHE[mm_dt] = build_nc(mm_dt)
    return _CACHE[mm_dt]


def _x_host(a):
    # [D, L] -> [chunk, p, dc, c]: value (ch, p, dc, c) = a[dc*128+p, ch*512+c]
    return np.ascontiguousarray(
        a.reshape(8, 128, 4, 512).transpose(2, 1, 0, 3)).astype(NP_MM)


def _w_host(w):
    # [D, CPC] -> [p, dc, c]
    return np.ascontiguousarray(
        w.reshape(8, 128, CPC).transpose(1, 0, 2)).astype(NP_MM)


def kernel(q, k, v, mask, Wq, bq, Wk, bk, Wv, bv, Wo, bo, _trace=False):
    nc = _get_nc(MM_DT)

    in_maps = []
    for c in range(NCORES):
        b = c // 4
        g = c % 4
        s = slice(g * CPC, (g + 1) * CPC)
        in_maps.append({
            "xq": _x_host(q[b].T),
            "xk": _x_host(k[b].T),
            "xv": _x_host(v[b].T),
            "wq": _w_host(Wq[:, s]),
            "wk": _w_host(Wk[:, s]),
            "wv": _w_host(Wv[:, s]),
            "wo": np.ascontiguousarray(Wo[s, :]).astype(NP_MM),
            "bq": np.ascontiguousarray(bq[s]).reshape(CPC, 1).astype(np.float32),
            "bk": np.ascontiguousarray(bk[s]).reshape(CPC, 1).astype(np.float32),
        })

    res = run_bass_kernel_spmd(nc, in_maps, list(range(NCORES)), trace=_trace)

    # host gather: out[b] = sum_g y_core(b,g) + (bo + bv @ Wo)
    const = (bo + bv.astype(np.float64) @ Wo.astype(np.float64)).astype(np.float64)
    out = np.zeros((B, L, D), np.float64)
    for c in range(NCORES):
        out[c // 4] += res.results[c]["y"].astype(np.float64)
    out += const[None, None, :]
    kernel.last_exec_time_ns = res.exec_time_ns
    return out.astype(np.float32)
